# revision 3
# baseline (speedup 1.0000x reference)
"""Trainium2 Bass kernel for nn_ActionNetwork (vq_codebook).

Data-parallel over 8 NeuronCores: core i processes batch element i
(8 timesteps from `states` + 8 from `next_states` = 16 images through the
conv encoder), then runs the per-sample head (affine/bilinear chain, MLP,
VQ distance + argmin) for its 8 samples. Host assembles idx/z and computes
the (exact) straight-through output and scalar loss.

Self-contained: only needs /opt/trn_rl_repo (the Bass/concourse stack) and
the axon-attached TRN2 cores.
"""
import sys, types
sys.path.insert(0, '/opt/trn_rl_repo')

try:  # register the NTFF profile hook (missing antenv.axon_hooks in image)
    from trn_agent_boot.trn_boot import _ntff_profile_via_ctypes
    _hook = _ntff_profile_via_ctypes('/opt/axon/libaxon_pjrt.so')
    _m = types.ModuleType('antenv.axon_hooks')
    _m.get_axon_ntff_profile_hook = lambda: _hook
    sys.modules.setdefault('antenv.axon_hooks', _m)
except Exception:
    pass

import numpy as np
import ml_dtypes

import concourse.bass as bass
import concourse.tile as tile
from concourse import bacc, mybir
from concourse.bass_utils import run_bass_kernel_spmd

f32 = mybir.dt.float32
bf16 = mybir.dt.bfloat16
AF = mybir.ActivationFunctionType
ALU = mybir.AluOpType
AX = mybir.AxisListType

# ---- problem geometry (hardcoded) ----
B, T, C, H, W = 8, 8, 64, 96, 96
C2 = 2 * C
NA, AD = 256, 64
NEG = 0.2
EPS = 1e-5
N_IMG = 2 * T                       # images per core: 8 states + 8 next_states

P1 = W + 1                          # wrap pitch rb1 = 97
INT1 = H * P1                       # 9312 f-space positions
B1 = P1 + 1                         # interior base = 98
XSZ = B1 + INT1 + P1 + 4            # x_pad / out1 size (covers +98 tap reach)

H2, W2 = 48, 48
P2 = W2 + 1                         # 49
INT2 = H2 * P2                      # 2352
B2 = P2 + 1                         # 50
PSZ = B2 + INT2 + P2 + 4            # pool_out / o3 size

CH1 = 512                           # conv1 chunk (flat)
CH2 = 4 * P1                        # conv2 chunk = 4 rows = 388 (pool-aligned)
CH3 = 512                           # rb2 chunk


def _ceil_div(a, b):
    return (a + b - 1) // b


def build_program():
    nc = bacc.Bacc("TRN2", target_bir_lowering=False, debug=False, num_devices=8)

    # ---- DRAM parameters ----
    s_img = nc.declare_dram_parameter("s_img", [N_IMG, C, H * W], bf16, isOutput=False)
    att_img = nc.declare_dram_parameter("att_img", [N_IMG, H * W], bf16, isOutput=False)

    w1a = nc.declare_dram_parameter("w1a", [128, 3, C2], bf16, isOutput=False)
    w1b = nc.declare_dram_parameter("w1b", [64, 3, C2], bf16, isOutput=False)
    ws1 = nc.declare_dram_parameter("ws1", [64, C2], bf16, isOutput=False)
    w2 = nc.declare_dram_parameter("w2", [128, 9, C2], bf16, isOutput=False)
    r2w1 = nc.declare_dram_parameter("r2w1", [128, 9, C], bf16, isOutput=False)
    r2w2 = nc.declare_dram_parameter("r2w2", [64, 9, C], bf16, isOutput=False)
    r2ws = nc.declare_dram_parameter("r2ws", [128, C], bf16, isOutput=False)
    b1p = nc.declare_dram_parameter("b1p", [C2, 1], f32, isOutput=False)
    b2sp = nc.declare_dram_parameter("b2sp", [C2, 1], f32, isOutput=False)
    r2b1p = nc.declare_dram_parameter("r2b1p", [C, 1], f32, isOutput=False)
    r2b2sp = nc.declare_dram_parameter("r2b2sp", [C, 1], f32, isOutput=False)

    affm = nc.declare_dram_parameter("affm", [C + 1, 4, C], f32, isOutput=False)
    pwm0 = nc.declare_dram_parameter("pwm0", [C, C2 * C], bf16, isOutput=False)
    pwm1 = nc.declare_dram_parameter("pwm1", [C2, C2 * C], bf16, isOutput=False)
    pwm2 = nc.declare_dram_parameter("pwm2", [C2, C2 * C], bf16, isOutput=False)
    pwm3 = nc.declare_dram_parameter("pwm3", [C2, C * C], bf16, isOutput=False)
    bn_sp = nc.declare_dram_parameter("bn_sp", [C2, 3], f32, isOutput=False)
    bn_bp = nc.declare_dram_parameter("bn_bp", [C2, 3], f32, isOutput=False)
    pb3p = nc.declare_dram_parameter("pb3p", [C, 1], f32, isOutput=False)
    mw1 = nc.declare_dram_parameter("mw1", [C, C2], f32, isOutput=False)
    mw2 = nc.declare_dram_parameter("mw2", [C2, C], f32, isOutput=False)
    mw3 = nc.declare_dram_parameter("mw3", [C, AD], f32, isOutput=False)
    mb1p = nc.declare_dram_parameter("mb1p", [C2, 1], f32, isOutput=False)
    mb2p = nc.declare_dram_parameter("mb2p", [C, 1], f32, isOutput=False)
    mb3p = nc.declare_dram_parameter("mb3p", [AD, 1], f32, isOutput=False)
    vqm = nc.declare_dram_parameter("vqm", [AD + 1, NA], f32, isOutput=False)
    iota_p = nc.declare_dram_parameter("iota_p", [T, NA], f32, isOutput=False)
    i8p = nc.declare_dram_parameter("i8p", [T, T], f32, isOutput=False)

    z_out = nc.declare_dram_parameter("z_out", [AD, T], f32, isOutput=True)
    idx_out = nc.declare_dram_parameter("idx_out", [T, 1], f32, isOutput=True)

    with tile.TileContext(nc) as tc:
        with tc.tile_pool(name="wpool", bufs=1) as wpool, \
             tc.tile_pool(name="big", bufs=1) as bigp, \
             tc.tile_pool(name="work", bufs=3) as work, \
             tc.tile_pool(name="attp", bufs=1) as attp, \
             tc.tile_pool(name="headp", bufs=2) as headp, \
             tc.tile_pool(name="cps", bufs=4, space="PSUM") as cps, \
             tc.tile_pool(name="hps", bufs=3, space="PSUM") as hps:

            # ---- resident weights ----
            w1a_t = wpool.tile([128, 3, C2], bf16)
            nc.sync.dma_start(out=w1a_t[:], in_=w1a[:])
            w1b_t = wpool.tile([64, 3, C2], bf16)
            nc.sync.dma_start(out=w1b_t[:], in_=w1b[:])
            ws1_t = wpool.tile([64, C2], bf16)
            nc.sync.dma_start(out=ws1_t[:], in_=ws1[:])
            w2_t = wpool.tile([128, 9, C2], bf16)
            nc.sync.dma_start(out=w2_t[:], in_=w2[:])
            r2w1_t = wpool.tile([128, 9, C], bf16)
            nc.sync.dma_start(out=r2w1_t[:], in_=r2w1[:])
            r2w2_t = wpool.tile([64, 9, C], bf16)
            nc.sync.dma_start(out=r2w2_t[:], in_=r2w2[:])
            r2ws_t = wpool.tile([128, C], bf16)
            nc.sync.dma_start(out=r2ws_t[:], in_=r2ws[:])
            b1_t = wpool.tile([C2, 1], f32)
            nc.sync.dma_start(out=b1_t[:], in_=b1p[:])
            b2s_t = wpool.tile([C2, 1], f32)
            nc.sync.dma_start(out=b2s_t[:], in_=b2sp[:])
            r2b1_t = wpool.tile([C, 1], f32)
            nc.sync.dma_start(out=r2b1_t[:], in_=r2b1p[:])
            r2b2s_t = wpool.tile([C, 1], f32)
            nc.sync.dma_start(out=r2b2s_t[:], in_=r2b2sp[:])

            # ---- big persistent encoder buffers ----
            xpads = [bigp.tile([128, XSZ], bf16, name=f"xpad{i}") for i in range(2)]
            atts = [bigp.tile([128, (H + 1) * W], bf16, name=f"att{i}") for i in range(2)]
            out1 = bigp.tile([128, XSZ], bf16)
            pool_out = bigp.tile([128, PSZ], bf16)
            o3 = bigp.tile([64, PSZ], bf16)
            o4 = bigp.tile([64, INT2], f32)
            x_all = bigp.tile([64, N_IMG], f32)

            for i in range(2):
                nc.vector.memset(xpads[i][:], 0.0)
                nc.vector.memset(atts[i][:], 0.0)
            nc.vector.memset(out1[:], 0.0)
            nc.vector.memset(pool_out[:], 0.0)
            nc.vector.memset(o3[:], 0.0)

            n_c1 = _ceil_div(INT1, CH1)          # 19
            n_c2 = INT1 // CH2                   # 24
            n_c3 = _ceil_div(INT2, CH3)          # 5

            # ---- encoder: 16 images ----
            for img in range(N_IMG):
                xp = xpads[img % 2]
                at = atts[img % 2]

                # load image: copy1 at B1 (parts 0-63), copy2 at B1-P1 (parts 64-127)
                src = s_img[img].rearrange("c (h w) -> c h w", w=W)
                dst1 = xp[0:64, B1:B1 + INT1].rearrange(
                    "p (h w) -> p h w", w=P1)[:, :, 0:W]
                nc.sync.dma_start(out=dst1, in_=src)
                dst2 = xp[64:128, B1 - P1:B1 - P1 + INT1].rearrange(
                    "p (h w) -> p h w", w=P1)[:, :, 0:W]
                nc.sync.dma_start(out=dst2, in_=src)
                # attention broadcast: parts 0-63 rows 1.., parts 64-127 rows 0..
                ain = att_img[img].partition_broadcast(64)
                nc.gpsimd.dma_start(out=at[0:64, W:(H + 1) * W], in_=ain)
                nc.gpsimd.dma_start(out=at[64:128, 0:H * W], in_=ain)
                # x *= att over rows -1..H-1 of the copy1 frame
                xv = xp[:, B1 - P1:B1 - P1 + (H + 1) * P1].rearrange(
                    "p (h w) -> p h w", w=P1)[:, :, 0:W]
                nc.vector.tensor_tensor(
                    xv, xv, at[:].rearrange("p (h w) -> p h w", w=W), ALU.mult)

                # ---- rb1 conv1: 6 matmuls per chunk -> out1 (Prelu) ----
                for c in range(n_c1):
                    f0 = c * CH1
                    n = min(CH1, INT1 - f0)
                    ps = cps.tile([C2, CH1], f32, tag="cps")
                    for dx in range(3):
                        off = -P1 - 1 + dx
                        nc.tensor.matmul(
                            ps[:, :n], lhsT=w1a_t[:, dx, :],
                            rhs=xp[:, B1 + f0 + off:B1 + f0 + off + n],
                            start=(dx == 0), stop=False)
                    for dx in range(3):
                        off = P1 - 1 + dx
                        nc.tensor.matmul(
                            ps[:, :n], lhsT=w1b_t[:, dx, :],
                            rhs=xp[0:64, B1 + f0 + off:B1 + f0 + off + n],
                            start=False, stop=(dx == 2))
                    nc.scalar.activation(
                        out=out1[:, B1 + f0:B1 + f0 + n], in_=ps[:, :n],
                        func=AF.Prelu, bias=b1_t[:], scale=1.0, alpha=NEG)
                # re-zero pad column of out1
                nc.vector.memset(
                    out1[:, B1 + W:B1 + W + H * P1].rearrange(
                        "p (h w) -> p h w", w=P1)[:, :, 0:1], 0.0)

                # ---- rb1 conv2 + shortcut; fused 2x2 avg pool ----
                for c in range(n_c2):
                    f0 = c * CH2
                    ps = cps.tile([C2, CH2], f32, tag="cps")
                    t = 0
                    for dy in range(3):
                        for dx in range(3):
                            off = (dy - 1) * P1 + (dx - 1)
                            nc.tensor.matmul(
                                ps[:], lhsT=w2_t[:, t, :],
                                rhs=out1[:, B1 + f0 + off:B1 + f0 + off + CH2],
                                start=(t == 0), stop=False)
                            t += 1
                    nc.tensor.matmul(
                        ps[:], lhsT=ws1_t[:],
                        rhs=xp[0:64, B1 + f0:B1 + f0 + CH2],
                        start=False, stop=True)
                    tmp = work.tile([128, CH2], bf16, tag="tmp")
                    nc.scalar.activation(out=tmp[:], in_=ps[:], func=AF.Prelu,
                                         bias=b2s_t[:], scale=1.0, alpha=NEG)
                    # pool: 4 rows (2 output rows)
                    tv = tmp[:].rearrange("p (a b w) -> p a b w", a=2, w=P1)
                    tmp2 = work.tile([128, 2, W], bf16, tag="tmp2")
                    nc.vector.tensor_tensor(
                        tmp2[:], tv[:, :, 0, 0:W], tv[:, :, 1, 0:W], ALU.add)
                    t2 = tmp2[:].rearrange("p a (w2 c) -> p a w2 c", c=2)
                    orow = 2 * c
                    pv = pool_out[:, B2 + orow * P2:B2 + orow * P2 + 2 * P2].rearrange(
                        "p (h w) -> p h w", w=P2)[:, :, 0:W2]
                    nc.vector.tensor_tensor(pv, t2[:, :, :, 0], t2[:, :, :, 1], ALU.add)

                # ---- rb2 conv1 -> o3 ----
                for c in range(n_c3):
                    f0 = c * CH3
                    n = min(CH3, INT2 - f0)
                    ps = cps.tile([C, CH3], f32, tag="cps")
                    t = 0
                    for dy in range(3):
                        for dx in range(3):
                            off = (dy - 1) * P2 + (dx - 1)
                            nc.tensor.matmul(
                                ps[:, :n], lhsT=r2w1_t[:, t, :],
                                rhs=pool_out[:, B2 + f0 + off:B2 + f0 + off + n],
                                start=(t == 0), stop=(t == 8))
                            t += 1
                    nc.scalar.activation(
                        out=o3[:, B2 + f0:B2 + f0 + n], in_=ps[:C, :n],
                        func=AF.Prelu, bias=r2b1_t[:], scale=1.0, alpha=NEG)
                nc.vector.memset(
                    o3[:, B2 + W2:B2 + W2 + H2 * P2].rearrange(
                        "p (h w) -> p h w", w=P2)[:, :, 0:1], 0.0)

                # ---- rb2 conv2 + shortcut -> o4 (flat) ----
                for c in range(n_c3):
                    f0 = c * CH3
                    n = min(CH3, INT2 - f0)
                    ps = cps.tile([C, CH3], f32, tag="cps")
                    t = 0
                    for dy in range(3):
                        for dx in range(3):
                            off = (dy - 1) * P2 + (dx - 1)
                            nc.tensor.matmul(
                                ps[:, :n], lhsT=r2w2_t[:, t, :],
                                rhs=o3[0:64, B2 + f0 + off:B2 + f0 + off + n],
                                start=(t == 0), stop=False)
                            t += 1
                    nc.tensor.matmul(
                        ps[:, :n], lhsT=r2ws_t[:],
                        rhs=pool_out[:, B2 + f0:B2 + f0 + n],
                        start=False, stop=True)
                    nc.scalar.activation(
                        out=o4[:, f0:f0 + n], in_=ps[:C, :n],
                        func=AF.Prelu, bias=r2b2s_t[:], scale=1.0, alpha=NEG)

                # global mean (sum; /2304 folded into head) over valid cols
                nc.vector.tensor_reduce(
                    out=x_all[:, img:img + 1],
                    in_=o4[:].rearrange("p (h w) -> p h w", w=P2)[:, :, 0:W2],
                    axis=AX.XY, op=ALU.add)

            # ================= head =================
            affm_t = wpool.tile([C + 1, 4, C], f32)
            nc.sync.dma_start(out=affm_t[:], in_=affm[:])
            bn_s_t = wpool.tile([C2, 3], f32)
            nc.sync.dma_start(out=bn_s_t[:], in_=bn_sp[:])
            bn_b_t = wpool.tile([C2, 3], f32)
            nc.sync.dma_start(out=bn_b_t[:], in_=bn_bp[:])
            pb3_t = wpool.tile([C, 1], f32)
            nc.sync.dma_start(out=pb3_t[:], in_=pb3p[:])
            mw1_t = wpool.tile([C, C2], f32)
            nc.sync.dma_start(out=mw1_t[:], in_=mw1[:])
            mw2_t = wpool.tile([C2, C], f32)
            nc.sync.dma_start(out=mw2_t[:], in_=mw2[:])
            mw3_t = wpool.tile([C, AD], f32)
            nc.sync.dma_start(out=mw3_t[:], in_=mw3[:])
            mb1_t = wpool.tile([C2, 1], f32)
            nc.sync.dma_start(out=mb1_t[:], in_=mb1p[:])
            mb2_t = wpool.tile([C, 1], f32)
            nc.sync.dma_start(out=mb2_t[:], in_=mb2p[:])
            mb3_t = wpool.tile([AD, 1], f32)
            nc.sync.dma_start(out=mb3_t[:], in_=mb3p[:])
            vqm_t = wpool.tile([AD + 1, NA], f32)
            nc.sync.dma_start(out=vqm_t[:], in_=vqm[:])
            iota_t = wpool.tile([T, NA], f32)
            nc.sync.dma_start(out=iota_t[:], in_=iota_p[:])
            i8_t = wpool.tile([T, T], f32)
            nc.sync.dma_start(out=i8_t[:], in_=i8p[:])
            big_t = wpool.tile([T, NA], f32)
            nc.vector.memset(big_t[:], 1e9)

            # x1 = x/2304 (+ ones row); p0 = nx/2304 (bf16)
            x1 = wpool.tile([C + 1, T], f32)
            nc.scalar.activation(out=x1[0:C, :], in_=x_all[:, 0:T],
                                 func=AF.Copy, scale=1.0 / (H2 * W2))
            nc.vector.memset(x1[C:C + 1, :], 1.0)

            prod = bigp.tile([T, C2 * C], f32)
            s_sb = headp.tile([T, C2], f32, tag="s_sb")

            p_cur = headp.tile([C2, T], bf16, tag="pfeat")
            nc.scalar.activation(out=p_cur[0:C, :], in_=x_all[:, T:2 * T],
                                 func=AF.Copy, scale=1.0 / (H2 * W2))

            pw_params = [pwm0, pwm1, pwm2, pwm3]
            for k in range(4):
                jk = C if k == 0 else C2
                ok = C2 if k < 3 else C
                oi = ok * C
                # a = x @ aff_k^T + b  -> [T, C]
                a_ps = hps.tile([T, C], f32, tag="hps")
                nc.tensor.matmul(a_ps[:], lhsT=x1[:], rhs=affm_t[:, k, :],
                                 start=True, stop=True)
                a_sb = headp.tile([T, C], f32, tag="a_sb")
                nc.scalar.copy(out=a_sb[:], in_=a_ps[:])

                # u chunks: [T, 512] = p^T @ PW chunk ; prod = u * a (bcast over o)
                n_str = oi // 2048
                for cs in range(n_str):
                    pw_t = work.tile([128, 2048], bf16, tag="pwstream")
                    nc.sync.dma_start(out=pw_t[0:jk, :],
                                      in_=pw_params[k][:, cs * 2048:(cs + 1) * 2048])
                    for cc in range(4):
                        off = cs * 2048 + cc * 512
                        u_ps = hps.tile([T, 512], f32, tag="hps")
                        nc.tensor.matmul(u_ps[:], lhsT=p_cur[0:jk, :],
                                         rhs=pw_t[0:jk, cc * 512:(cc + 1) * 512],
                                         start=True, stop=True)
                        nc.vector.tensor_tensor(
                            prod[:, off:off + 512].rearrange(
                                "p (a b) -> p a b", b=C),
                            u_ps[:].rearrange("p (a b) -> p a b", b=C),
                            a_sb[:, None, :].to_broadcast((T, 8, C)),
                            ALU.mult)
                # s[b, o] = sum_i prod
                nc.vector.tensor_reduce(
                    out=s_sb[:, 0:ok],
                    in_=prod[:, 0:oi].rearrange("p (o i) -> p o i", i=C),
                    axis=AX.X, op=ALU.add)
                # transpose s -> [ok, T]
                t_ps = hps.tile([C2, T], f32, tag="hps")
                nc.tensor.matmul(t_ps[0:ok, :], lhsT=s_sb[:, 0:ok], rhs=i8_t[:],
                                 start=True, stop=True)
                if k < 3:
                    p_cur = headp.tile([C2, T], bf16, tag="pfeat")
                    nc.scalar.activation(
                        out=p_cur[:], in_=t_ps[0:C2, :], func=AF.Prelu,
                        bias=bn_b_t[:, k:k + 1], scale=bn_s_t[:, k:k + 1], alpha=NEG)
                else:
                    d1 = headp.tile([C, T], f32, tag="d1")
                    # d = (t + pb3) - x/2304
                    nc.vector.scalar_tensor_tensor(
                        out=d1[:], in0=t_ps[0:C, :], scalar=pb3_t[:],
                        in1=x1[0:C, :], op0=ALU.add, op1=ALU.subtract)

            # mlp
            z1_ps = hps.tile([C2, T], f32, tag="hps")
            nc.tensor.matmul(z1_ps[:], lhsT=mw1_t[:], rhs=d1[:], start=True, stop=True)
            z1 = headp.tile([C2, T], f32, tag="z1")
            nc.scalar.activation(out=z1[:], in_=z1_ps[:], func=AF.Prelu,
                                 bias=mb1_t[:], scale=1.0, alpha=NEG)
            z2_ps = hps.tile([C, T], f32, tag="hps")
            nc.tensor.matmul(z2_ps[:], lhsT=mw2_t[:], rhs=z1[:], start=True, stop=True)
            z2 = headp.tile([C, T], f32, tag="z2")
            nc.scalar.activation(out=z2[:], in_=z2_ps[:], func=AF.Prelu,
                                 bias=mb2_t[:], scale=1.0, alpha=NEG)
            z3_ps = hps.tile([AD, T], f32, tag="hps")
            nc.tensor.matmul(z3_ps[:], lhsT=mw3_t[:], rhs=z2[:], start=True, stop=True)
            z1s = headp.tile([AD + 1, T], f32, tag="z1s")
            nc.scalar.activation(out=z1s[0:AD, :], in_=z3_ps[:], func=AF.Identity,
                                 bias=mb3_t[:], scale=1.0)
            nc.vector.memset(z1s[AD:AD + 1, :], 1.0)

            # vq: dd = ee - 2 z.e ; argmin
            dd_ps = hps.tile([T, NA], f32, tag="hps")
            nc.tensor.matmul(dd_ps[:], lhsT=z1s[:], rhs=vqm_t[:], start=True, stop=True)
            m_t = headp.tile([T, 1], f32, tag="m_t")
            nc.vector.tensor_reduce(out=m_t[:], in_=dd_ps[:], axis=AX.X, op=ALU.min)
            mask_t = headp.tile([T, NA], mybir.dt.uint8, tag="mask")
            nc.vector.tensor_scalar(mask_t[:], dd_ps[:], m_t[:], None, ALU.is_le)
            sel_t = headp.tile([T, NA], f32, tag="sel")
            nc.vector.select(sel_t[:], mask_t[:], iota_t[:], big_t[:])
            idx_t = headp.tile([T, 1], f32, tag="idx")
            nc.vector.tensor_reduce(out=idx_t[:], in_=sel_t[:], axis=AX.X, op=ALU.min)

            nc.sync.dma_start(out=idx_out[:], in_=idx_t[:])
            nc.sync.dma_start(out=z_out[:], in_=z1s[0:AD, :])

    nc.compile()
    return nc


def prepare_inputs(inputs):
    """Host-side weight folding + per-core shard maps."""
    gi = lambda k: np.asarray(inputs[k], np.float32)
    bnscale = np.float32(1.0 / np.sqrt(1.0 + EPS))
    bf = ml_dtypes.bfloat16

    def conv_w(w, scale=1.0):
        # w (O, I, 3, 3) -> per-tap lhsT [I, O], scaled
        return (np.asarray(w, np.float32) * scale)

    # rb1 conv1 (C->C2), folded bn scale
    w1 = conv_w(gi('rb1_w1'), gi('rb1_g1')[:, None, None, None] * bnscale)
    w1a = np.zeros((128, 3, C2), np.float32)
    w1b = np.zeros((64, 3, C2), np.float32)
    for dx in range(3):
        w1a[0:64, dx, :] = w1[:, :, 0, dx].T
        w1a[64:128, dx, :] = w1[:, :, 1, dx].T
        w1b[:, dx, :] = w1[:, :, 2, dx].T
    b1 = (gi('rb1_b1') * gi('rb1_g1') * bnscale + gi('rb1_be1')).reshape(C2, 1)

    w2w = conv_w(gi('rb1_w2'), gi('rb1_g2')[:, None, None, None] * bnscale)
    w2 = np.zeros((128, 9, C2), np.float32)
    t = 0
    for dy in range(3):
        for dx in range(3):
            w2[:, t, :] = w2w[:, :, dy, dx].T
            t += 1
    b2s = (gi('rb1_b2') * gi('rb1_g2') * bnscale + gi('rb1_be2')
           + gi('rb1_bs')).reshape(C2, 1)
    ws1 = gi('rb1_ws')[:, :, 0, 0].T  # [C, C2]

    # rb2 conv1 consumes pool output -> fold 0.25
    r2w1w = conv_w(gi('rb2_w1'), gi('rb2_g1')[:, None, None, None] * bnscale * 0.25)
    r2w1 = np.zeros((128, 9, C), np.float32)
    t = 0
    for dy in range(3):
        for dx in range(3):
            r2w1[:, t, :] = r2w1w[:, :, dy, dx].T
            t += 1
    r2b1 = (gi('rb2_b1') * gi('rb2_g1') * bnscale + gi('rb2_be1')).reshape(C, 1)
    r2w2w = conv_w(gi('rb2_w2'), gi('rb2_g2')[:, None, None, None] * bnscale)
    r2w2 = np.zeros((64, 9, C), np.float32)
    t = 0
    for dy in range(3):
        for dx in range(3):
            r2w2[:, t, :] = r2w2w[:, :, dy, dx].T
            t += 1
    r2b2s = (gi('rb2_b2') * gi('rb2_g2') * bnscale + gi('rb2_be2')
             + gi('rb2_bs')).reshape(C, 1)
    r2ws = (gi('rb2_ws')[:, :, 0, 0] * 0.25).T  # [C2, C]

    affm = np.zeros((C + 1, 4, C), np.float32)
    aw = gi('aff_w')
    ab = gi('aff_b')
    for k in range(4):
        affm[0:C, k, :] = aw[k].T
        affm[C, k, :] = ab[k]

    pwms = []
    for k, nm in enumerate(['proj_w0', 'proj_w1', 'proj_w2', 'proj_w3']):
        pw = gi(nm)                        # (o, i, j)
        pwm = np.transpose(pw, (2, 0, 1)).reshape(pw.shape[2], -1)  # [j, o*i]
        pwms.append(pwm.astype(bf))
    bn_s = np.zeros((C2, 3), np.float32)
    bn_b = np.zeros((C2, 3), np.float32)
    for k in range(3):
        sk = gi('bn_g')[k] * bnscale
        bn_s[:, k] = sk
        bn_b[:, k] = sk * gi(f'proj_b{k}') + gi('bn_b')[k]
    pb3 = gi('proj_b3').reshape(C, 1)

    mw1 = (gi('mlp_w1') * (gi('mlp_g1') * bnscale)[:, None]).T  # [C, C2]
    mb1 = (gi('mlp_b1') * gi('mlp_g1') * bnscale + gi('mlp_be1')).reshape(C2, 1)
    mw2 = (gi('mlp_w2') * (gi('mlp_g2') * bnscale)[:, None]).T  # [C2, C]
    mb2 = (gi('mlp_b2') * gi('mlp_g2') * bnscale + gi('mlp_be2')).reshape(C, 1)
    mw3 = gi('mlp_w3').T                                        # [C, AD]
    mb3 = gi('mlp_b3').reshape(AD, 1)

    emb = gi('emb')
    vqm = np.zeros((AD + 1, NA), np.float32)
    vqm[0:AD, :] = -2.0 * emb.T
    vqm[AD, :] = (emb * emb).sum(1)
    iota = np.tile(np.arange(NA, dtype=np.float32), (T, 1))
    i8 = np.eye(T, dtype=np.float32)

    shared = {
        'w1a': w1a.astype(bf), 'w1b': w1b.astype(bf), 'ws1': ws1.astype(bf),
        'w2': w2.astype(bf), 'r2w1': r2w1.astype(bf), 'r2w2': r2w2.astype(bf),
        'r2ws': r2ws.astype(bf), 'b1p': b1, 'b2sp': b2s, 'r2b1p': r2b1,
        'r2b2sp': r2b2s, 'affm': affm,
        'pwm0': pwms[0], 'pwm1': pwms[1], 'pwm2': pwms[2], 'pwm3': pwms[3],
        'bn_sp': bn_s, 'bn_bp': bn_b, 'pb3p': pb3,
        'mw1': mw1, 'mw2': mw2, 'mw3': mw3,
        'mb1p': mb1, 'mb2p': mb2, 'mb3p': mb3,
        'vqm': vqm, 'iota_p': iota, 'i8p': i8,
    }

    states = np.asarray(inputs['states'], np.float32)
    natt = np.asarray(inputs['next_states_attention'], np.float32)
    satt = np.asarray(inputs['states_attention'], np.float32)
    nstates = np.asarray(inputs['next_states'], np.float32)

    in_maps = []
    for core in range(8):
        s_core = np.concatenate(
            [states[core].reshape(T, C, H * W),
             nstates[core].reshape(T, C, H * W)], axis=0).astype(bf)
        a_core = np.concatenate(
            [satt[core].reshape(T, H * W), natt[core].reshape(T, H * W)],
            axis=0).astype(bf)
        m = dict(shared)
        m['s_img'] = s_core
        m['att_img'] = a_core
        in_maps.append(m)
    return in_maps


_PROGRAM = None


def kernel(**inputs):
    global _PROGRAM
    if _PROGRAM is None:
        _PROGRAM = build_program()
    in_maps = prepare_inputs(inputs)
    res = run_bass_kernel_spmd(_PROGRAM, in_maps, core_ids=list(range(8)))
    kernel.last_result = res

    z = np.zeros((B * T, AD), np.float32)
    idx = np.zeros(B * T, np.int64)
    for core in range(8):
        z[core * T:(core + 1) * T, :] = res.results[core]['z_out'].T
        idx[core * T:(core + 1) * T] = np.rint(
            res.results[core]['idx_out'].ravel()).astype(np.int64)

    emb = np.asarray(inputs['emb'], np.float32)
    zq = emb[idx]
    BETA = 0.25
    vq_loss = np.float32(BETA * np.mean((zq - z) ** 2, dtype=np.float32)
                         + np.mean((zq - z) ** 2, dtype=np.float32))
    zq_st = (z + (zq - z)).reshape(B, T, AD)
    return vq_loss, zq_st, idx.reshape(B, T).astype(np.int32)


# revision 10
# speedup vs baseline: 1.1786x; 1.1786x over previous
"""Trainium2 Bass kernel for nn_ActionNetwork (vq_codebook).

Data-parallel over 8 NeuronCores: core i processes batch element i
(8 timesteps from `states` + 8 from `next_states` = 16 images through the
conv encoder), then runs the per-sample head (affine/bilinear chain, MLP,
VQ distance + argmin) for its 8 samples. Host assembles idx/z and computes
the (exact) straight-through output and scalar loss.

Self-contained: only needs /opt/trn_rl_repo (the Bass/concourse stack) and
the axon-attached TRN2 cores.
"""
import sys, types
sys.path.insert(0, '/opt/trn_rl_repo')

try:  # register the NTFF profile hook (missing antenv.axon_hooks in image)
    from trn_agent_boot.trn_boot import _ntff_profile_via_ctypes
    _hook = _ntff_profile_via_ctypes('/opt/axon/libaxon_pjrt.so')
    _m = types.ModuleType('antenv.axon_hooks')
    _m.get_axon_ntff_profile_hook = lambda: _hook
    sys.modules.setdefault('antenv.axon_hooks', _m)
except Exception:
    pass

import numpy as np
import ml_dtypes

import concourse.bass as bass
import concourse.tile as tile
from concourse import bacc, mybir
from concourse.bass_utils import run_bass_kernel_spmd

f32 = mybir.dt.float32
bf16 = mybir.dt.bfloat16
AF = mybir.ActivationFunctionType
ALU = mybir.AluOpType
AX = mybir.AxisListType

# ---- problem geometry (hardcoded) ----
B, T, C, H, W = 8, 8, 64, 96, 96
C2 = 2 * C
NA, AD = 256, 64
NEG = 0.2
EPS = 1e-5
N_IMG = 2 * T                       # images per core: 8 states + 8 next_states

P1 = W + 1                          # wrap pitch rb1 = 97
INT1 = H * P1                       # 9312 f-space positions
B1 = P1 + 1                         # interior base = 98
XSZ = B1 + INT1 + P1 + 4            # x_pad / out1 size (covers +98 tap reach)

H2, W2 = 48, 48
P2 = W2 + 1                         # 49
INT2 = H2 * P2                      # 2352
B2 = P2 + 1                         # 50
PSZ = B2 + INT2 + P2 + 4            # pool_out / o3 size

CH1 = 512                           # conv1 chunk (flat)
CH2 = 4 * P1                        # conv2 chunk = 4 rows = 388 (pool-aligned)
CH3 = 512                           # rb2 chunk


def _ceil_div(a, b):
    return (a + b - 1) // b


def build_program():
    nc = bacc.Bacc("TRN2", target_bir_lowering=False, debug=False, num_devices=8)

    # ---- DRAM parameters ----
    s_img = nc.declare_dram_parameter("s_img", [N_IMG, C, INT1], bf16, isOutput=False)
    att_img = nc.declare_dram_parameter("att_img", [N_IMG, 128, (H + 1) * W], bf16, isOutput=False)

    w1a = nc.declare_dram_parameter("w1a", [128, 3, C2], bf16, isOutput=False)
    w1b = nc.declare_dram_parameter("w1b", [64, 3, C2], bf16, isOutput=False)
    ws1 = nc.declare_dram_parameter("ws1", [64, C2], bf16, isOutput=False)
    w2 = nc.declare_dram_parameter("w2", [128, 9, C2], bf16, isOutput=False)
    r2w1 = nc.declare_dram_parameter("r2w1", [128, 9, C], bf16, isOutput=False)
    r2w2 = nc.declare_dram_parameter("r2w2", [64, 9, C], bf16, isOutput=False)
    r2ws = nc.declare_dram_parameter("r2ws", [128, C], bf16, isOutput=False)
    b1p = nc.declare_dram_parameter("b1p", [C2, 1], f32, isOutput=False)
    b2sp = nc.declare_dram_parameter("b2sp", [C2, 1], f32, isOutput=False)
    r2b1p = nc.declare_dram_parameter("r2b1p", [C, 1], f32, isOutput=False)
    r2b2sp = nc.declare_dram_parameter("r2b2sp", [C, 1], f32, isOutput=False)

    affm = nc.declare_dram_parameter("affm", [C + 1, 4, C], f32, isOutput=False)
    pwm0 = nc.declare_dram_parameter("pwm0", [C, C2 * C], bf16, isOutput=False)
    pwm1 = nc.declare_dram_parameter("pwm1", [C2, C2 * C], bf16, isOutput=False)
    pwm2 = nc.declare_dram_parameter("pwm2", [C2, C2 * C], bf16, isOutput=False)
    pwm3 = nc.declare_dram_parameter("pwm3", [C2, C * C], bf16, isOutput=False)
    bn_sp = nc.declare_dram_parameter("bn_sp", [C2, 3], f32, isOutput=False)
    bn_bp = nc.declare_dram_parameter("bn_bp", [C2, 3], f32, isOutput=False)
    pb3p = nc.declare_dram_parameter("pb3p", [C, 1], f32, isOutput=False)
    mw1 = nc.declare_dram_parameter("mw1", [C, C2], f32, isOutput=False)
    mw2 = nc.declare_dram_parameter("mw2", [C2, C], f32, isOutput=False)
    mw3 = nc.declare_dram_parameter("mw3", [C, AD], f32, isOutput=False)
    mb1p = nc.declare_dram_parameter("mb1p", [C2, 1], f32, isOutput=False)
    mb2p = nc.declare_dram_parameter("mb2p", [C, 1], f32, isOutput=False)
    mb3p = nc.declare_dram_parameter("mb3p", [AD, 1], f32, isOutput=False)
    vqm = nc.declare_dram_parameter("vqm", [AD + 1, NA], f32, isOutput=False)
    iota_p = nc.declare_dram_parameter("iota_p", [T, NA], f32, isOutput=False)
    i8p = nc.declare_dram_parameter("i8p", [T, T], f32, isOutput=False)

    z_out = nc.declare_dram_parameter("z_out", [AD, T], f32, isOutput=True)
    idx_out = nc.declare_dram_parameter("idx_out", [T, 1], f32, isOutput=True)

    with tile.TileContext(nc) as tc:
        with tc.tile_pool(name="wpool", bufs=1) as wpool, \
             tc.tile_pool(name="big", bufs=1) as bigp, \
             tc.tile_pool(name="work", bufs=3) as work, \
             tc.tile_pool(name="attp", bufs=1) as attp, \
             tc.tile_pool(name="headp", bufs=2) as headp, \
             tc.tile_pool(name="pwsp", bufs=6) as pwsp, \
             tc.tile_pool(name="cps", bufs=5, space="PSUM") as cps, \
             tc.tile_pool(name="hps", bufs=3, space="PSUM") as hps:

            # ---- resident weights ----
            w1a_t = wpool.tile([128, 3, C2], bf16)
            nc.sync.dma_start(out=w1a_t[:], in_=w1a[:])
            w1b_t = wpool.tile([64, 3, C2], bf16)
            nc.sync.dma_start(out=w1b_t[:], in_=w1b[:])
            ws1_t = wpool.tile([64, C2], bf16)
            nc.sync.dma_start(out=ws1_t[:], in_=ws1[:])
            w2_t = wpool.tile([128, 9, C2], bf16)
            nc.sync.dma_start(out=w2_t[:], in_=w2[:])
            r2w1_t = wpool.tile([128, 9, C], bf16)
            nc.sync.dma_start(out=r2w1_t[:], in_=r2w1[:])
            r2w2_t = wpool.tile([64, 9, C], bf16)
            nc.sync.dma_start(out=r2w2_t[:], in_=r2w2[:])
            r2ws_t = wpool.tile([128, C], bf16)
            nc.sync.dma_start(out=r2ws_t[:], in_=r2ws[:])
            b1_t = wpool.tile([C2, 1], f32)
            nc.sync.dma_start(out=b1_t[:], in_=b1p[:])
            b2s_t = wpool.tile([C2, 1], f32)
            nc.sync.dma_start(out=b2s_t[:], in_=b2sp[:])
            r2b1_t = wpool.tile([C, 1], f32)
            nc.sync.dma_start(out=r2b1_t[:], in_=r2b1p[:])
            r2b2s_t = wpool.tile([C, 1], f32)
            nc.sync.dma_start(out=r2b2s_t[:], in_=r2b2sp[:])

            # ---- big persistent encoder buffers ----
            xpads = [bigp.tile([128, XSZ], bf16, name=f"xpad{i}") for i in range(2)]
            atts = [bigp.tile([128, (H + 1) * W], bf16, name=f"att{i}") for i in range(2)]
            out1 = bigp.tile([128, XSZ], bf16)
            pool_outs = [bigp.tile([128, PSZ], bf16, name=f"pool{i}") for i in range(2)]
            o3 = bigp.tile([64, PSZ], bf16)
            o4 = bigp.tile([64, INT2], f32)
            x_all = bigp.tile([64, N_IMG], f32)

            def zero_pads(tile_, base, pitch, size):
                # top+corner, pad columns (multiples of pitch), tail
                nc.vector.memset(tile_[:, 0:base], 0.0)
                h = size // pitch - 1
                nc.vector.memset(
                    tile_[:, pitch:pitch + h * pitch].rearrange(
                        "p (h w) -> p h w", w=pitch)[:, :, 0:1], 0.0)
                nc.vector.memset(tile_[:, size - 2 * pitch:size], 0.0)

            for i in range(2):
                zero_pads(xpads[i], B1, P1, XSZ)

            zero_pads(out1, B1, P1, XSZ)
            zero_pads(pool_outs[0], B2, P2, PSZ)
            zero_pads(pool_outs[1], B2, P2, PSZ)
            zero_pads(o3, B2, P2, PSZ)

            n_c1 = _ceil_div(INT1, CH1)          # 19
            n_c2 = INT1 // CH2                   # 24
            n_c3 = _ceil_div(INT2, CH3)          # 5

            # ---- encoder: 16 images, rb2 pipelined one image behind ----
            def conv12(img):
                xp = xpads[img % 2]
                at = atts[img % 2]
                pool_out = pool_outs[img % 2]

                # load image: copy1 at B1 (parts 0-63), copy2 at B1-P1 (parts 64-127)
                src = s_img[img].rearrange("c (h w) -> c h w", w=W)
                dst1 = xp[0:64, B1:B1 + INT1].rearrange(
                    "p (h w) -> p h w", w=P1)[:, :, 0:W]
                nc.sync.dma_start(out=dst1, in_=src)
                dst2 = xp[64:128, B1 - P1:B1 - P1 + INT1].rearrange(
                    "p (h w) -> p h w", w=P1)[:, :, 0:W]
                nc.sync.dma_start(out=dst2, in_=src)
                # attention (host pre-broadcast, both partition frames)
                nc.gpsimd.dma_start(out=at[:], in_=att_img[img])
                # x *= att over rows -1..H-1 of the copy1 frame
                xv = xp[:, B1 - P1:B1 - P1 + (H + 1) * P1].rearrange(
                    "p (h w) -> p h w", w=P1)[:, :, 0:W]
                nc.vector.tensor_tensor(
                    xv, xv, at[:].rearrange("p (h w) -> p h w", w=W), ALU.mult)

                # ---- rb1 conv1: 6 matmuls per chunk -> out1 (Prelu) ----
                for c in range(n_c1):
                    f0 = c * CH1
                    n = min(CH1, INT1 - f0)
                    ps = cps.tile([C2, CH1], f32, tag="cps")
                    for dx in range(3):
                        off = -P1 - 1 + dx
                        nc.tensor.matmul(
                            ps[:, :n], lhsT=w1a_t[:, dx, :],
                            rhs=xp[:, B1 + f0 + off:B1 + f0 + off + n],
                            start=(dx == 0), stop=False)
                    for dx in range(3):
                        off = P1 - 1 + dx
                        nc.tensor.matmul(
                            ps[:, :n], lhsT=w1b_t[:, dx, :],
                            rhs=xp[0:64, B1 + f0 + off:B1 + f0 + off + n],
                            start=False, stop=(dx == 2))
                    nc.scalar.activation(
                        out=out1[:, B1 + f0:B1 + f0 + n], in_=ps[:, :n],
                        func=AF.Prelu, bias=b1_t[:], scale=1.0, alpha=NEG)
                # re-zero pad column of out1
                nc.vector.memset(
                    out1[:, B1 + W:B1 + W + H * P1].rearrange(
                        "p (h w) -> p h w", w=P1)[:, :, 0:1], 0.0)

                # ---- rb1 conv2 + shortcut; fused 2x2 avg pool ----
                for c in range(n_c2):
                    f0 = c * CH2
                    ps = cps.tile([C2, CH2], f32, tag="cps")
                    t = 0
                    for dy in range(3):
                        for dx in range(3):
                            off = (dy - 1) * P1 + (dx - 1)
                            nc.tensor.matmul(
                                ps[:], lhsT=w2_t[:, t, :],
                                rhs=out1[:, B1 + f0 + off:B1 + f0 + off + CH2],
                                start=(t == 0), stop=False)
                            t += 1
                    nc.tensor.matmul(
                        ps[:], lhsT=ws1_t[:],
                        rhs=xp[0:64, B1 + f0:B1 + f0 + CH2],
                        start=False, stop=True)
                    tmp = work.tile([128, CH2], bf16, tag="tmp")
                    nc.scalar.activation(out=tmp[:], in_=ps[:], func=AF.Prelu,
                                         bias=b2s_t[:], scale=1.0, alpha=NEG)
                    # pool: 4 rows (2 output rows)
                    tv = tmp[:].rearrange("p (a b w) -> p a b w", a=2, w=P1)
                    tmp2 = work.tile([128, 2, W], bf16, tag="tmp2")
                    nc.vector.tensor_tensor(
                        tmp2[:], tv[:, :, 0, 0:W], tv[:, :, 1, 0:W], ALU.add)
                    t2 = tmp2[:].rearrange("p a (w2 c) -> p a w2 c", c=2)
                    orow = 2 * c
                    pv = pool_out[:, B2 + orow * P2:B2 + orow * P2 + 2 * P2].rearrange(
                        "p (h w) -> p h w", w=P2)[:, :, 0:W2]
                    nc.vector.tensor_tensor(pv, t2[:, :, :, 0], t2[:, :, :, 1], ALU.add)

            def rb2(img):
                pool_out = pool_outs[img % 2]
                # ---- rb2 conv1 -> o3 ----
                for c in range(n_c3):
                    f0 = c * CH3
                    n = min(CH3, INT2 - f0)
                    ps = cps.tile([C, CH3], f32, tag="cps")
                    t = 0
                    for dy in range(3):
                        for dx in range(3):
                            off = (dy - 1) * P2 + (dx - 1)
                            nc.tensor.matmul(
                                ps[:, :n], lhsT=r2w1_t[:, t, :],
                                rhs=pool_out[:, B2 + f0 + off:B2 + f0 + off + n],
                                start=(t == 0), stop=(t == 8))
                            t += 1
                    nc.scalar.activation(
                        out=o3[:, B2 + f0:B2 + f0 + n], in_=ps[:C, :n],
                        func=AF.Prelu, bias=r2b1_t[:], scale=1.0, alpha=NEG)
                nc.vector.memset(
                    o3[:, B2 + W2:B2 + W2 + H2 * P2].rearrange(
                        "p (h w) -> p h w", w=P2)[:, :, 0:1], 0.0)

                # ---- rb2 conv2 + shortcut -> o4 (flat) ----
                for c in range(n_c3):
                    f0 = c * CH3
                    n = min(CH3, INT2 - f0)
                    ps = cps.tile([C, CH3], f32, tag="cps")
                    t = 0
                    for dy in range(3):
                        for dx in range(3):
                            off = (dy - 1) * P2 + (dx - 1)
                            nc.tensor.matmul(
                                ps[:, :n], lhsT=r2w2_t[:, t, :],
                                rhs=o3[0:64, B2 + f0 + off:B2 + f0 + off + n],
                                start=(t == 0), stop=False)
                            t += 1
                    nc.tensor.matmul(
                        ps[:, :n], lhsT=r2ws_t[:],
                        rhs=pool_out[:, B2 + f0:B2 + f0 + n],
                        start=False, stop=True)
                    nc.scalar.activation(
                        out=o4[:, f0:f0 + n], in_=ps[:C, :n],
                        func=AF.Prelu, bias=r2b2s_t[:], scale=1.0, alpha=NEG)

                # global mean (sum; /2304 folded into head) over valid cols
                nc.vector.tensor_reduce(
                    out=x_all[:, img:img + 1],
                    in_=o4[:].rearrange("p (h w) -> p h w", w=P2)[:, :, 0:W2],
                    axis=AX.XY, op=ALU.add)

            for img in range(N_IMG):
                conv12(img)
                if img > 0:
                    rb2(img - 1)
            rb2(N_IMG - 1)

            # ================= head =================
            affm_t = wpool.tile([C + 1, 4, C], f32)
            nc.sync.dma_start(out=affm_t[:], in_=affm[:])
            bn_s_t = wpool.tile([C2, 3], f32)
            nc.sync.dma_start(out=bn_s_t[:], in_=bn_sp[:])
            bn_b_t = wpool.tile([C2, 3], f32)
            nc.sync.dma_start(out=bn_b_t[:], in_=bn_bp[:])
            pb3_t = wpool.tile([C, 1], f32)
            nc.sync.dma_start(out=pb3_t[:], in_=pb3p[:])
            mw1_t = wpool.tile([C, C2], f32)
            nc.sync.dma_start(out=mw1_t[:], in_=mw1[:])
            mw2_t = wpool.tile([C2, C], f32)
            nc.sync.dma_start(out=mw2_t[:], in_=mw2[:])
            mw3_t = wpool.tile([C, AD], f32)
            nc.sync.dma_start(out=mw3_t[:], in_=mw3[:])
            mb1_t = wpool.tile([C2, 1], f32)
            nc.sync.dma_start(out=mb1_t[:], in_=mb1p[:])
            mb2_t = wpool.tile([C, 1], f32)
            nc.sync.dma_start(out=mb2_t[:], in_=mb2p[:])
            mb3_t = wpool.tile([AD, 1], f32)
            nc.sync.dma_start(out=mb3_t[:], in_=mb3p[:])
            vqm_t = wpool.tile([AD + 1, NA], f32)
            nc.sync.dma_start(out=vqm_t[:], in_=vqm[:])
            iota_t = wpool.tile([T, NA], f32)
            nc.sync.dma_start(out=iota_t[:], in_=iota_p[:])
            i8_t = wpool.tile([T, T], f32)
            nc.sync.dma_start(out=i8_t[:], in_=i8p[:])
            big_t = wpool.tile([T, NA], f32)
            nc.vector.memset(big_t[:], 1e9)

            # x1 = x/2304 (+ ones row); p0 = nx/2304 (bf16)
            x1 = wpool.tile([C + 1, T], f32)
            nc.scalar.activation(out=x1[0:C, :], in_=x_all[:, 0:T],
                                 func=AF.Copy, scale=1.0 / (H2 * W2))
            nc.vector.memset(x1[C:C + 1, :], 1.0)

            prod = bigp.tile([T, C2 * C], f32)
            s_sb = headp.tile([T, C2], f32, tag="s_sb")

            p_cur = headp.tile([C2, T], bf16, tag="pfeat")
            nc.scalar.activation(out=p_cur[0:C, :], in_=x_all[:, T:2 * T],
                                 func=AF.Copy, scale=1.0 / (H2 * W2))

            pw_params = [pwm0, pwm1, pwm2, pwm3]
            for k in range(4):
                jk = C if k == 0 else C2
                ok = C2 if k < 3 else C
                oi = ok * C
                # a = x @ aff_k^T + b  -> [T, C]
                a_ps = hps.tile([T, C], f32, tag="hps")
                nc.tensor.matmul(a_ps[:], lhsT=x1[:], rhs=affm_t[:, k, :],
                                 start=True, stop=True)
                a_sb = headp.tile([T, C], f32, tag="a_sb")
                nc.scalar.copy(out=a_sb[:], in_=a_ps[:])

                # u chunks: [T, 512] = p^T @ PW chunk ; prod = u * a (bcast over o)
                n_str = oi // 2048
                for cs in range(n_str):
                    pw_t = pwsp.tile([128, 2048], bf16, tag="pwstream")
                    nc.sync.dma_start(out=pw_t[0:jk, :],
                                      in_=pw_params[k][:, cs * 2048:(cs + 1) * 2048])
                    for cc in range(4):
                        off = cs * 2048 + cc * 512
                        u_ps = hps.tile([T, 512], f32, tag="hps")
                        nc.tensor.matmul(u_ps[:], lhsT=p_cur[0:jk, :],
                                         rhs=pw_t[0:jk, cc * 512:(cc + 1) * 512],
                                         start=True, stop=True)
                        nc.vector.tensor_tensor(
                            prod[:, off:off + 512].rearrange(
                                "p (a b) -> p a b", b=C),
                            u_ps[:].rearrange("p (a b) -> p a b", b=C),
                            a_sb[:, None, :].to_broadcast((T, 8, C)),
                            ALU.mult)
                        oc0 = off // C
                        nc.vector.tensor_reduce(
                            out=s_sb[:, oc0:oc0 + 8],
                            in_=prod[:, off:off + 512].rearrange(
                                "p (o i) -> p o i", i=C),
                            axis=AX.X, op=ALU.add)
                # transpose s -> [ok, T]
                t_ps = hps.tile([C2, T], f32, tag="hps")
                nc.tensor.matmul(t_ps[0:ok, :], lhsT=s_sb[:, 0:ok], rhs=i8_t[:],
                                 start=True, stop=True)
                if k < 3:
                    p_cur = headp.tile([C2, T], bf16, tag="pfeat")
                    nc.scalar.activation(
                        out=p_cur[:], in_=t_ps[0:C2, :], func=AF.Prelu,
                        bias=bn_b_t[:, k:k + 1], scale=bn_s_t[:, k:k + 1], alpha=NEG)
                else:
                    d1 = headp.tile([C, T], f32, tag="d1")
                    # d = (t + pb3) - x/2304
                    nc.vector.scalar_tensor_tensor(
                        out=d1[:], in0=t_ps[0:C, :], scalar=pb3_t[:],
                        in1=x1[0:C, :], op0=ALU.add, op1=ALU.subtract)

            # mlp
            z1_ps = hps.tile([C2, T], f32, tag="hps")
            nc.tensor.matmul(z1_ps[:], lhsT=mw1_t[:], rhs=d1[:], start=True, stop=True)
            z1 = headp.tile([C2, T], f32, tag="z1")
            nc.scalar.activation(out=z1[:], in_=z1_ps[:], func=AF.Prelu,
                                 bias=mb1_t[:], scale=1.0, alpha=NEG)
            z2_ps = hps.tile([C, T], f32, tag="hps")
            nc.tensor.matmul(z2_ps[:], lhsT=mw2_t[:], rhs=z1[:], start=True, stop=True)
            z2 = headp.tile([C, T], f32, tag="z2")
            nc.scalar.activation(out=z2[:], in_=z2_ps[:], func=AF.Prelu,
                                 bias=mb2_t[:], scale=1.0, alpha=NEG)
            z3_ps = hps.tile([AD, T], f32, tag="hps")
            nc.tensor.matmul(z3_ps[:], lhsT=mw3_t[:], rhs=z2[:], start=True, stop=True)
            z1s = headp.tile([AD + 1, T], f32, tag="z1s")
            nc.scalar.activation(out=z1s[0:AD, :], in_=z3_ps[:], func=AF.Identity,
                                 bias=mb3_t[:], scale=1.0)
            nc.vector.memset(z1s[AD:AD + 1, :], 1.0)

            # vq: dd = ee - 2 z.e ; argmin
            dd_ps = hps.tile([T, NA], f32, tag="hps")
            nc.tensor.matmul(dd_ps[:], lhsT=z1s[:], rhs=vqm_t[:], start=True, stop=True)
            m_t = headp.tile([T, 1], f32, tag="m_t")
            nc.vector.tensor_reduce(out=m_t[:], in_=dd_ps[:], axis=AX.X, op=ALU.min)
            mask_t = headp.tile([T, NA], mybir.dt.uint8, tag="mask")
            nc.vector.tensor_scalar(mask_t[:], dd_ps[:], m_t[:], None, ALU.is_le)
            sel_t = headp.tile([T, NA], f32, tag="sel")
            nc.vector.select(sel_t[:], mask_t[:], iota_t[:], big_t[:])
            idx_t = headp.tile([T, 1], f32, tag="idx")
            nc.vector.tensor_reduce(out=idx_t[:], in_=sel_t[:], axis=AX.X, op=ALU.min)

            nc.sync.dma_start(out=idx_out[:], in_=idx_t[:])
            nc.sync.dma_start(out=z_out[:], in_=z1s[0:AD, :])

    nc.compile()
    return nc


def prepare_inputs(inputs):
    """Host-side weight folding + per-core shard maps."""
    gi = lambda k: np.asarray(inputs[k], np.float32)
    bnscale = np.float32(1.0 / np.sqrt(1.0 + EPS))
    bf = ml_dtypes.bfloat16

    def conv_w(w, scale=1.0):
        # w (O, I, 3, 3) -> per-tap lhsT [I, O], scaled
        return (np.asarray(w, np.float32) * scale)

    # rb1 conv1 (C->C2), folded bn scale
    w1 = conv_w(gi('rb1_w1'), gi('rb1_g1')[:, None, None, None] * bnscale)
    w1a = np.zeros((128, 3, C2), np.float32)
    w1b = np.zeros((64, 3, C2), np.float32)
    for dx in range(3):
        w1a[0:64, dx, :] = w1[:, :, 0, dx].T
        w1a[64:128, dx, :] = w1[:, :, 1, dx].T
        w1b[:, dx, :] = w1[:, :, 2, dx].T
    b1 = (gi('rb1_b1') * gi('rb1_g1') * bnscale + gi('rb1_be1')).reshape(C2, 1)

    w2w = conv_w(gi('rb1_w2'), gi('rb1_g2')[:, None, None, None] * bnscale)
    w2 = np.zeros((128, 9, C2), np.float32)
    t = 0
    for dy in range(3):
        for dx in range(3):
            w2[:, t, :] = w2w[:, :, dy, dx].T
            t += 1
    b2s = (gi('rb1_b2') * gi('rb1_g2') * bnscale + gi('rb1_be2')
           + gi('rb1_bs')).reshape(C2, 1)
    ws1 = gi('rb1_ws')[:, :, 0, 0].T  # [C, C2]

    # rb2 conv1 consumes pool output -> fold 0.25
    r2w1w = conv_w(gi('rb2_w1'), gi('rb2_g1')[:, None, None, None] * bnscale * 0.25)
    r2w1 = np.zeros((128, 9, C), np.float32)
    t = 0
    for dy in range(3):
        for dx in range(3):
            r2w1[:, t, :] = r2w1w[:, :, dy, dx].T
            t += 1
    r2b1 = (gi('rb2_b1') * gi('rb2_g1') * bnscale + gi('rb2_be1')).reshape(C, 1)
    r2w2w = conv_w(gi('rb2_w2'), gi('rb2_g2')[:, None, None, None] * bnscale)
    r2w2 = np.zeros((64, 9, C), np.float32)
    t = 0
    for dy in range(3):
        for dx in range(3):
            r2w2[:, t, :] = r2w2w[:, :, dy, dx].T
            t += 1
    r2b2s = (gi('rb2_b2') * gi('rb2_g2') * bnscale + gi('rb2_be2')
             + gi('rb2_bs')).reshape(C, 1)
    r2ws = (gi('rb2_ws')[:, :, 0, 0] * 0.25).T  # [C2, C]

    affm = np.zeros((C + 1, 4, C), np.float32)
    aw = gi('aff_w')
    ab = gi('aff_b')
    for k in range(4):
        affm[0:C, k, :] = aw[k].T
        affm[C, k, :] = ab[k]

    pwms = []
    for k, nm in enumerate(['proj_w0', 'proj_w1', 'proj_w2', 'proj_w3']):
        pw = gi(nm)                        # (o, i, j)
        pwm = np.transpose(pw, (2, 0, 1)).reshape(pw.shape[2], -1)  # [j, o*i]
        pwms.append(pwm.astype(bf))
    bn_s = np.zeros((C2, 3), np.float32)
    bn_b = np.zeros((C2, 3), np.float32)
    for k in range(3):
        sk = gi('bn_g')[k] * bnscale
        bn_s[:, k] = sk
        bn_b[:, k] = sk * gi(f'proj_b{k}') + gi('bn_b')[k]
    pb3 = gi('proj_b3').reshape(C, 1)

    mw1 = (gi('mlp_w1') * (gi('mlp_g1') * bnscale)[:, None]).T  # [C, C2]
    mb1 = (gi('mlp_b1') * gi('mlp_g1') * bnscale + gi('mlp_be1')).reshape(C2, 1)
    mw2 = (gi('mlp_w2') * (gi('mlp_g2') * bnscale)[:, None]).T  # [C2, C]
    mb2 = (gi('mlp_b2') * gi('mlp_g2') * bnscale + gi('mlp_be2')).reshape(C, 1)
    mw3 = gi('mlp_w3').T                                        # [C, AD]
    mb3 = gi('mlp_b3').reshape(AD, 1)

    emb = gi('emb')
    vqm = np.zeros((AD + 1, NA), np.float32)
    vqm[0:AD, :] = -2.0 * emb.T
    vqm[AD, :] = (emb * emb).sum(1)
    iota = np.tile(np.arange(NA, dtype=np.float32), (T, 1))
    i8 = np.eye(T, dtype=np.float32)

    shared = {
        'w1a': w1a.astype(bf), 'w1b': w1b.astype(bf), 'ws1': ws1.astype(bf),
        'w2': w2.astype(bf), 'r2w1': r2w1.astype(bf), 'r2w2': r2w2.astype(bf),
        'r2ws': r2ws.astype(bf), 'b1p': b1, 'b2sp': b2s, 'r2b1p': r2b1,
        'r2b2sp': r2b2s, 'affm': affm,
        'pwm0': pwms[0], 'pwm1': pwms[1], 'pwm2': pwms[2], 'pwm3': pwms[3],
        'bn_sp': bn_s, 'bn_bp': bn_b, 'pb3p': pb3,
        'mw1': mw1, 'mw2': mw2, 'mw3': mw3,
        'mb1p': mb1, 'mb2p': mb2, 'mb3p': mb3,
        'vqm': vqm, 'iota_p': iota, 'i8p': i8,
    }

    states = np.asarray(inputs['states'], np.float32)
    natt = np.asarray(inputs['next_states_attention'], np.float32)
    satt = np.asarray(inputs['states_attention'], np.float32)
    nstates = np.asarray(inputs['next_states'], np.float32)

    in_maps = []
    for core in range(8):
        s_raw = np.concatenate(
            [states[core].reshape(T, C, H, W),
             nstates[core].reshape(T, C, H, W)], axis=0)
        s_pad = np.zeros((N_IMG, C, H, P1), np.float32)
        s_pad[:, :, :, 0:W] = s_raw
        s_core = s_pad.reshape(N_IMG, C, INT1).astype(bf)
        a_raw = np.concatenate(
            [satt[core].reshape(T, H, W), natt[core].reshape(T, H, W)], axis=0)
        a_core = np.zeros((N_IMG, 128, (H + 1) * W), np.float32)
        a_core[:, 0:64, W:] = a_raw.reshape(N_IMG, 1, H * W)
        a_core[:, 64:128, 0:H * W] = a_raw.reshape(N_IMG, 1, H * W)
        a_core = a_core.astype(bf)
        m = dict(shared)
        m['s_img'] = s_core
        m['att_img'] = a_core
        in_maps.append(m)
    return in_maps


_PROGRAM = None


def kernel(**inputs):
    global _PROGRAM
    if _PROGRAM is None:
        _PROGRAM = build_program()
    in_maps = prepare_inputs(inputs)
    res = run_bass_kernel_spmd(_PROGRAM, in_maps, core_ids=list(range(8)))
    kernel.last_result = res

    z = np.zeros((B * T, AD), np.float32)
    idx = np.zeros(B * T, np.int64)
    for core in range(8):
        z[core * T:(core + 1) * T, :] = res.results[core]['z_out'].T
        idx[core * T:(core + 1) * T] = np.rint(
            res.results[core]['idx_out'].ravel()).astype(np.int64)

    emb = np.asarray(inputs['emb'], np.float32)
    zq = emb[idx]
    BETA = 0.25
    vq_loss = np.float32(BETA * np.mean((zq - z) ** 2, dtype=np.float32)
                         + np.mean((zq - z) ** 2, dtype=np.float32))
    zq_st = (z + (zq - z)).reshape(B, T, AD)
    return vq_loss, zq_st, idx.reshape(B, T).astype(np.int32)
